# revision 1
# baseline (speedup 1.0000x reference)
"""Trainium2 Bass kernel for BatchGATConv (GAT message passing).

Strategy (8 NeuronCores, SPMD):
  - Edges are partitioned by destination-node range: core c owns dst nodes
    [c*2500, (c+1)*2500). Host sorts edges by dst and packs them, per
    128-node "node tile", into 128-edge chunks (padded with dstl=-1 slots).
  - Each core (replicated work) projects all node features ft = feat @ W and
    per-node logits el/er (attn vectors folded into the weight matrix on the
    host), writing a gather table row per (node, batch):
    g[2n+b] = [ft(n,b) 256 | el(n,b) 4 | er(n,b) 4].
  - Per 128-edge chunk: one indirect-DMA gather of the row PAIR g[2*src]
    (528 floats), edge logits e = leaky(el_src + er_dst) with er_dst
    selected via a one-hot matmul, ex = exp(e) (no max-subtraction needed:
    logits are O(5), exp is safe in fp32 and the softmax is mathematically
    identical), messages m = ft*ex, then a one-hot segment-sum matmul
    accumulating numerator and denominator in PSUM across the tile's chunks.
    Per node tile: out = leaky(num/denom), written contiguously.
  - Matmuls run as float32r (fp32 bits, single-pass PE streaming mode).
"""

import numpy as np

try:
    import concourse.bass as bass
except ImportError:  # pragma: no cover
    import sys

    sys.path.insert(0, "/opt/trn_rl_repo")
    import concourse.bass as bass

import concourse.bacc as bacc
import concourse.mybir as mybir
import concourse.tile as tile
from concourse.bass_utils import run_bass_kernel_spmd

P = 128
F32 = mybir.dt.float32
F32R = mybir.dt.float32r
I32 = mybir.dt.int32

# problem constants
N, B, DIN, H, D, E = 20000, 2, 128, 4, 64, 320000
NEG = 0.2
NCORES = 8
HB = H * B  # 8 logits per node/edge
HD = H * D  # 256 projected feats per (n, b)
FT = B * H * D  # 512 projected feats per node
WC = HD + 2 * H  # 264 = [W | W.attn_l | W.attn_r] columns; also g-row width
GW = 2 * WC  # 528 = gathered row-pair width

USE_F32R = False
# dtype for matmul operands: f32r streams through the PE in a single pass
# (vs 2 half-speed passes for plain fp32). The BIR verifier requires every
# f32r-matmul operand to be produced with dtype f32r, so the operand tiles
# (and the DRAM tensors DMA'd straight into them) are declared f32r; numpy
# bits are identical fp32 either way.
MMDT = F32R if USE_F32R else F32


def _mm(ap):
    return ap


def _host_prep(src, dst, n_nodes, n_cores):
    """Sort edges by dst; pack per (core, node-tile) into 128-edge chunks.

    Returns (K, idx_T, dstl_T, nid_T):
      K: per-node-tile chunk count (shared across cores; program structure)
      idx_T[c]:  [P, sum(K)] int32, gather row (= 2*src) per chunk slot
      dstl_T[c]: [P, sum(K)] float32, dst-local index in [0,128) or -1 pad
      nid_T[c]:  [P, nt] int32, gather row (= 2*node) per tile partition
    """
    npc = n_nodes // n_cores
    nt = (npc + P - 1) // P
    order = np.argsort(dst, kind="stable")
    ss = np.ascontiguousarray(src[order]).astype(np.int64)
    ds = np.ascontiguousarray(dst[order]).astype(np.int64)

    lows = np.array(
        [c * npc + t * P for c in range(n_cores) for t in range(nt + 1)],
        dtype=np.int64,
    )
    lows = np.minimum(lows, n_nodes)
    bounds = np.searchsorted(ds, lows).reshape(n_cores, nt + 1)
    cnts = bounds[:, 1:] - bounds[:, :-1]  # [n_cores, nt]

    K = np.maximum(1, -(-cnts.max(axis=0) // P)).astype(np.int64)  # per tile
    SK = int(K.sum())
    offs = np.concatenate([[0], np.cumsum(K)[:-1]])

    idx_all = np.zeros((n_cores, SK, P), np.int32)
    dstl_all = np.full((n_cores, SK, P), -1.0, np.float32)
    for c in range(n_cores):
        for t in range(nt):
            e0 = bounds[c, t]
            cnt = int(cnts[c, t])
            if cnt == 0:
                continue
            s = np.arange(cnt)
            rows = offs[t] + s // P
            cols = s % P
            idx_all[c, rows, cols] = 2 * ss[e0 : e0 + cnt]
            dstl_all[c, rows, cols] = (ds[e0 : e0 + cnt] - (c * npc + t * P)).astype(
                np.float32
            )

    nid_all = np.zeros((n_cores, nt, P), np.int32)
    base = np.arange(P)
    for c in range(n_cores):
        for t in range(nt):
            nid_all[c, t] = 2 * np.minimum(c * npc + t * P + base, n_nodes - 1)

    idx_T = [np.ascontiguousarray(idx_all[c].T) for c in range(n_cores)]
    dstl_T = [np.ascontiguousarray(dstl_all[c].T) for c in range(n_cores)]
    nid_T = [np.ascontiguousarray(nid_all[c].T) for c in range(n_cores)]
    # transposed one-hot selector per chunk: st[chunk, dst_local, slot] = 1
    st_T = []
    for c in range(n_cores):
        st = np.zeros((SK, P, P), np.float32)
        ch, dl, sl = np.nonzero(dstl_all[c][:, None, :] == np.arange(P)[None, :, None])
        st[ch, dl, sl] = 1.0
        st_T.append(np.ascontiguousarray(st.reshape(SK * P, P)))
    return list(map(int, K)), idx_T, dstl_T, nid_T, st_T


def _build(n_nodes, npc, K):
    """Build the SPMD Bass program (identical for all cores)."""
    R = n_nodes * B
    RT = (R + P - 1) // P
    nt = len(K)
    SK = sum(K)

    nc = bacc.Bacc(trn_type="TRN2", num_swdge_queues=2)
    featT = nc.dram_tensor("featT", [DIN, R], MMDT, kind="ExternalInput")
    wmat = nc.dram_tensor("wmat", [DIN, WC], MMDT, kind="ExternalInput")
    idxd = nc.dram_tensor("idx", [P, SK], I32, kind="ExternalInput")
    dstld = nc.dram_tensor("dstl", [P, SK], F32, kind="ExternalInput")
    nidd = nc.dram_tensor("nid", [P, nt], I32, kind="ExternalInput")
    sttd = nc.dram_tensor("stt", [SK * P, P], MMDT, kind="ExternalInput")
    outd = nc.dram_tensor("out", [npc, FT], F32, kind="ExternalOutput")
    g = nc.dram_tensor("gtab", [R, WC], F32)

    with tile.TileContext(nc) as tc:
        with (
            tc.tile_pool(name="const", bufs=1) as cp,
            tc.tile_pool(name="proj", bufs=8) as pp,
            tc.tile_pool(name="projps", bufs=2, space="PSUM") as ppp,
            tc.tile_pool(name="agg", bufs=12) as ag,
            tc.tile_pool(name="aggo", bufs=4) as og,
            tc.tile_pool(name="accps", bufs=2, space="PSUM") as psp,
            tc.tile_pool(name="smallps", bufs=2, space="PSUM") as psp1,
        ):
            # Resident constants. Matmul operands are routed through DVE
            # copies so matmul waits merge into a single DVE semaphore (the
            # fp32/f32r Matmult ISA struct has one sync-wait slot; Bacc's
            # event-semaphore pass splits the rest, but fewer is faster).
            w_sb0 = cp.tile([DIN, WC], MMDT)
            nc.sync.dma_start(w_sb0[:], wmat[:])
            w_sb = cp.tile([DIN, WC], MMDT)
            nc.vector.tensor_copy(w_sb[:], w_sb0[:])
            iota_i = cp.tile([P, P], I32)
            nc.gpsimd.iota(iota_i[:], pattern=[[1, P]], base=0, channel_multiplier=0)
            iota_f = cp.tile([P, P], F32)
            nc.vector.tensor_copy(iota_f[:], iota_i[:])
            idx_res = cp.tile([P, SK], I32)
            nc.sync.dma_start(idx_res[:], idxd[:])
            dstl_res = cp.tile([P, SK], F32)
            nc.sync.dma_start(dstl_res[:], dstld[:])
            nid_res = cp.tile([P, nt], I32)
            nc.sync.dma_start(nid_res[:], nidd[:])
            zero0 = cp.tile([P, FT], F32)
            nc.gpsimd.memset(zero0[:], 0.0)
            zero_sb = cp.tile([P, FT], F32)
            nc.vector.tensor_copy(zero_sb[:], zero0[:])

            # ---- projection: g[2n+b] = [ft(n,b) | el(n,b) | er(n,b)] ----
            for it in range(RT):
                r0 = it * P
                rows = min(P, R - r0)
                qs = (nc.sync, nc.scalar, nc.gpsimd)
                ftT = pp.tile([DIN, rows], MMDT, tag="ftT")
                qs[it % 3].dma_start(ftT[:], featT[:, r0 : r0 + rows])
                po = ppp.tile([rows, WC], F32, tag="po")
                nc.tensor.matmul(
                    po[:], lhsT=_mm(ftT[:]), rhs=_mm(w_sb[:]), start=True, stop=True
                )
                pout = pp.tile([rows, WC], F32, tag="pout")
                nc.vector.tensor_copy(pout[:], po[:])
                qs[(it + 1) % 3].dma_start(out=g[r0 : r0 + rows, :], in_=pout[:])

            # ---- aggregation: per node tile, segment softmax + weighted sum ----
            off = 0
            for t in range(nt):
                tn = min(P, npc - t * P)
                # er for the tile's own nodes: indirect gather reads
                # CONTIGUOUS source bytes, so fetch the 268-element span from
                # er(n,b0) at col 260 of row 2n through er(n,b1) at the end of
                # row 2n+1; er(b0) lands at flat cols 0:4, er(b1) at 264:268.
                er_t0 = og.tile([P, B, WC], F32, tag="er_t0")
                nc.gpsimd.indirect_dma_start(
                    out=er_t0[:].rearrange("p b c -> p (b c)")[:, 0 : WC + H],
                    out_offset=None,
                    in_=g[:],
                    in_offset=bass.IndirectOffsetOnAxis(
                        ap=nid_res[:, t : t + 1], axis=0
                    ),
                    element_offset=HD + H,
                )
                er_t = og.tile([P, HB], MMDT, tag="er_t")
                nc.vector.tensor_copy(
                    er_t[:].rearrange("p (b h) -> p b h", b=B),
                    er_t0[:, :, 0:H],
                )
                acc_a = psp.tile([P, WC], F32, tag="acca")
                acc_b = psp.tile([P, HD], F32, tag="accb")
                # zero-clear via DVE so PSUM bank-WAW/WAR waits stay off the
                # accumulating matmuls; adding onto DVE-written zeros is exact
                # whether or not the write cleared the has_written bits.
                nc.vector.tensor_copy(acc_a[:], zero_sb[:, :WC])
                nc.vector.tensor_copy(acc_b[:], zero_sb[:, :HD])
                for k in range(K[t]):
                    col = off + k
                    gt = ag.tile([P, GW], F32, tag="gt")
                    nc.gpsimd.indirect_dma_start(
                        out=gt[:],
                        out_offset=None,
                        in_=g[:],
                        in_offset=bass.IndirectOffsetOnAxis(
                            ap=idx_res[:, col : col + 1], axis=0
                        ),
                    )
                    gt3 = gt[:].rearrange("p (b c) -> p b c", b=B)
                    S = ag.tile([P, P], MMDT, tag="S")
                    nc.vector.tensor_scalar(
                        out=S[:],
                        in0=iota_f[:],
                        scalar1=dstl_res[:, col : col + 1],
                        scalar2=None,
                        op0=mybir.AluOpType.is_equal,
                    )
                    S_T = ag.tile([P, P], MMDT, tag="ST")
                    nc.sync.dma_start(S_T[:], sttd[col * P : (col + 1) * P, :])
                    eep = psp1.tile([P, HB], F32, tag="eep")
                    nc.tensor.matmul(
                        eep[:], lhsT=_mm(S_T[:]), rhs=_mm(er_t[:]), start=True, stop=True
                    )
                    lg = ag.tile([P, HB], F32, tag="lg")
                    nc.vector.tensor_add(
                        lg[:].rearrange("p (b h) -> p b h", b=B),
                        gt3[:, :, HD : HD + H],
                        eep[:].rearrange("p (b h) -> p b h", b=B),
                    )
                    l1 = ag.tile([P, HB], F32, tag="l1")
                    nc.vector.scalar_tensor_tensor(
                        out=l1[:],
                        in0=lg[:],
                        scalar=NEG,
                        in1=lg[:],
                        op0=mybir.AluOpType.mult,
                        op1=mybir.AluOpType.max,
                    )
                    # m_ext = [m(b0) 256 | exs 8 | m(b1) 256]: the first 264
                    # columns and last 256 columns feed two >=256-wide f32r
                    # matmuls, folding the denominator into the first bank.
                    m_ext = ag.tile([P, 2, WC], MMDT, tag="m")
                    nc.scalar.activation(
                        m_ext[:, 0, HD:HD + HB], l1[:],
                        mybir.ActivationFunctionType.Exp,
                    )
                    exs_v = m_ext[:, 0, HD:HD + HB]
                    nc.vector.tensor_tensor(
                        out=m_ext[:, :, 0:HD].rearrange("p b (h d) -> p b h d", d=D),
                        in0=gt3[:, :, 0:HD].rearrange("p b (h d) -> p b h d", d=D),
                        in1=exs_v.rearrange("p (b h) -> p b h", b=B)[:, :, :, None]
                        .to_broadcast([P, B, H, D]),
                        op=mybir.AluOpType.mult,
                    )
                    nc.tensor.matmul(
                        acc_a[:],
                        lhsT=_mm(S[:]),
                        rhs=_mm(m_ext[:].rearrange("p b c -> p (b c)")[:, 0:WC]),
                        start=False,
                        stop=(k == K[t] - 1),
                        skip_group_check=True,
                    )
                    nc.tensor.matmul(
                        acc_b[:],
                        lhsT=_mm(S[:]),
                        rhs=_mm(m_ext[:, 1, 0:HD]),
                        start=False,
                        stop=(k == K[t] - 1),
                        skip_group_check=True,
                    )
                off += K[t]
                dsum = og.tile([P, HB], F32, tag="dsum")
                nc.vector.tensor_scalar_add(dsum[:], acc_a[:, HD:HD + HB], 1e-30)
                rcp = og.tile([P, HB, 1], F32, tag="rcp")
                nc.vector.reciprocal(rcp[:, :, 0], dsum[:])
                o1 = og.tile([P, HB, D], F32, tag="o1")
                nc.vector.tensor_tensor(
                    out=o1[:, 0:H, :],
                    in0=acc_a[:, 0:HD].rearrange("p (h d) -> p h d", d=D),
                    in1=rcp[:, 0:H].to_broadcast([P, H, D]),
                    op=mybir.AluOpType.mult,
                )
                nc.vector.tensor_tensor(
                    out=o1[:, H:HB, :],
                    in0=acc_b[:].rearrange("p (h d) -> p h d", d=D),
                    in1=rcp[:, H:HB].to_broadcast([P, H, D]),
                    op=mybir.AluOpType.mult,
                )
                o3 = og.tile([P, FT], F32, tag="o3")
                nc.vector.scalar_tensor_tensor(
                    out=o3[:].rearrange("p (h d) -> p h d", d=D),
                    in0=o1[:],
                    scalar=NEG,
                    in1=o1[:],
                    op0=mybir.AluOpType.mult,
                    op1=mybir.AluOpType.max,
                )
                nc.sync.dma_start(out=outd[t * P : t * P + tn, :], in_=o3[:tn, :])

    nc.compile()
    _check_matmul_waits(nc)
    return nc


def _check_matmul_waits(nc):
    """fp32/f32r Matmult has a single ISA sync-wait slot; walrus codegen
    hard-fails on more. Catch it at build time."""
    bad = []
    for bb in nc.main_func.blocks:
        for ins in bb.instructions:
            if type(ins).__name__ == "InstMatmult":
                si = ins.sync_info
                nw = len(si.on_wait) if si is not None and si.on_wait else 0
                if nw > 1:
                    bad.append((ins.name, [w.ant_name for w in si.on_wait]))
    if bad:
        raise RuntimeError(f"matmuls with >1 sync wait: {bad[:10]} (n={len(bad)})")


def _make_inputs(feat, W, attn_l, attn_r, src, dst, n_nodes, n_cores):
    feat = np.asarray(feat, dtype=np.float32)
    W = np.asarray(W, dtype=np.float32)
    attn_l = np.asarray(attn_l, dtype=np.float32)
    attn_r = np.asarray(attn_r, dtype=np.float32)
    src = np.asarray(src)
    dst = np.asarray(dst)

    featT = np.ascontiguousarray(feat.reshape(n_nodes * B, DIN).T)
    Wl = (W.reshape(DIN, H, D) * attn_l[None]).sum(-1).astype(np.float32)
    Wr = (W.reshape(DIN, H, D) * attn_r[None]).sum(-1).astype(np.float32)
    wmat = np.ascontiguousarray(np.concatenate([W, Wl, Wr], axis=1))

    K, idx_T, dstl_T, nid_T, st_T = _host_prep(src, dst, n_nodes, n_cores)
    in_maps = [
        {
            "featT": featT,
            "wmat": wmat,
            "idx": idx_T[c],
            "dstl": dstl_T[c],
            "nid": nid_T[c],
            "stt": st_T[c],
        }
        for c in range(n_cores)
    ]
    return K, in_maps


_CACHE = {}


def kernel(feat, W, attn_l, attn_r, src, dst):
    K, in_maps = _make_inputs(feat, W, attn_l, attn_r, src, dst, N, NCORES)
    key = tuple(K)
    if key not in _CACHE:
        _CACHE[key] = _build(N, N // NCORES, K)
    nc = _CACHE[key]
    res = run_bass_kernel_spmd(nc, in_maps, list(range(NCORES))).results
    out = np.concatenate([res[c]["out"] for c in range(NCORES)], axis=0)
    return np.ascontiguousarray(out.reshape(N, B, H, D))


if __name__ == "__main__":
    rng = np.random.default_rng(0)
    feat = rng.standard_normal((N, B, DIN), dtype=np.float32)
    W = rng.standard_normal((DIN, H * D), dtype=np.float32) / np.sqrt(DIN)
    al = rng.standard_normal((H, D), dtype=np.float32) * 0.1
    ar = rng.standard_normal((H, D), dtype=np.float32) * 0.1
    src = rng.integers(0, N, E).astype(np.int32)
    dst = rng.integers(0, N, E).astype(np.int32)
    out = kernel(feat=feat, W=W, attn_l=al, attn_r=ar, src=src, dst=dst)
    print(out.shape, out.dtype, np.abs(out).mean())



# revision 6
# speedup vs baseline: 1.3827x; 1.3827x over previous
"""Trainium2 Bass kernel for BatchGATConv (GAT message passing).

Strategy (8 NeuronCores, SPMD, dst-partitioned):
  - Host: in-degree-sort each core's 2500 dst nodes into 20 tiles of 128 so
    every tile has near-uniform degree; chunk k of a tile holds the k-th
    in-edge of each of the tile's 128 nodes (slot-aligned, so dst-local ==
    partition and no one-hot matmuls are needed). Padded slots point at a
    dedicated pad row whose attention logit is -150 (=> exp == 0).
  - Projection (replicated on all cores, bf16): g[pos] row pair
    [ft(b0)|el|er | ft(b1)|el|er | pad] with attn_l/attn_r folded into the
    weight matrix on the host; 1024-row blocks, batched DMAs.
  - Aggregation per tile: ONE dma_gather fetches all (K+1)*128 row pairs
    (the +1 chunk gathers the tile's own rows for er_dst); logits
    lg = el_src + er_dst, leaky, exp on ACT; messages ft*ex scaled in-place
    on DVE; PSUM-accumulated via identity matmuls (per-partition segment sum);
    denominator via DVE reduce; out = leaky(num/den).
"""

import numpy as np

try:
    import concourse.bass as bass
except ImportError:  # pragma: no cover
    import sys

    sys.path.insert(0, "/opt/trn_rl_repo")
    import concourse.bass as bass

import ml_dtypes
import concourse.bacc as bacc
import concourse.mybir as mybir
import concourse.tile as tile
from concourse.bass_utils import run_bass_kernel_spmd

P = 128
F32 = mybir.dt.float32
BF16 = mybir.dt.bfloat16
I32 = mybir.dt.int32
I16 = mybir.dt.int16

# problem constants
N, B, DIN, H, D, E = 20000, 2, 128, 4, 64, 320000
NEG = 0.2
NCORES = 8
NPC = N // NCORES  # 2500 dst nodes per core
NT = 20  # node tiles per core
NPOSC = NT * P  # 2560 positions per core (60 pad positions)
NPOS = NCORES * NPOSC  # 20480 global positions
PADROW = NPOS  # pad pair row index
GROWS = NPOS + 2
WC = 264  # per-(n,b) block: [ft 256 | el 4 | er 4]
HD = H * D  # 256
FT = B * HD  # 512
RW = 640  # g row width (2*WC + 112 pad; 1280B, %256 for dma_gather)
PAD_EL = -150.0
QBLK = 1024  # projection rows per block
NBLK = NPOS // QBLK  # 20


def _host_prep(src, dst):
    """Degree-sort nodes per core; build per-core wrapped int16 gather tables.

    Returns (K, idxs_all, pos2node):
      K: per-tile chunk count, shared across cores (max in-degree in tile)
      idxs_all[c]: [16, sum((K+1)*8)] int16 wrapped gather indices
      pos2node: [NPOS] original node id per position (-1 = pad)
    """
    deg = np.bincount(dst, minlength=N).astype(np.int64)
    pos2node = np.full(NPOS, -1, np.int64)
    for c in range(NCORES):
        ids = np.arange(c * NPC, (c + 1) * NPC)
        order = np.argsort(-deg[ids], kind="stable")
        pos2node[c * NPOSC : c * NPOSC + NPC] = ids[order]
    node2pos = np.empty(N, np.int64)
    real = pos2node >= 0
    node2pos[pos2node[real]] = np.nonzero(real)[0]

    first = np.arange(NCORES)[:, None] * NPOSC + np.arange(NT)[None, :] * P
    K = np.maximum(deg[pos2node[first]].max(axis=0), 1)  # [NT]

    dpos = node2pos[dst]
    spos = node2pos[src]
    order_e = np.argsort(dpos, kind="stable")
    ds = dpos[order_e]
    ss = spos[order_e]
    starts = np.searchsorted(ds, np.arange(NPOS + 1))
    k_e = np.arange(E) - starts[ds]

    idxs_all = []
    for c in range(NCORES):
        cols = []
        for t in range(NT):
            Kt = int(K[t])
            base = c * NPOSC + t * P
            sl = np.full((Kt + 1) * P, PADROW, np.int64)
            e0, e1 = starts[base], starts[base + P]
            pp_ = ds[e0:e1] - base
            sl[k_e[e0:e1] * P + pp_] = ss[e0:e1]
            sl[Kt * P : (Kt + 1) * P] = base + np.arange(P)
            cols.append(sl.reshape(-1, 16).T)
        idxs_all.append(
            np.ascontiguousarray(
                np.tile(np.concatenate(cols, axis=1), (8, 1))
            ).astype(np.int16)
        )
    return list(map(int, K)), idxs_all, pos2node


def _build(K, stage='all'):
    S8 = sum((k + 1) * 8 for k in K)
    KP1M = max(K) + 1
    AF = mybir.ActivationFunctionType
    OP = mybir.AluOpType

    nc = bacc.Bacc(trn_type="TRN2", num_swdge_queues=2)
    featT = nc.dram_tensor("featT", [DIN, B * NPOS], BF16, kind="ExternalInput")
    wmat = nc.dram_tensor("wmat", [DIN, WC], BF16, kind="ExternalInput")
    idxsd = nc.dram_tensor("idxs", [128, S8], I16, kind="ExternalInput")
    outd = nc.dram_tensor("out", [NPOSC, FT], F32, kind="ExternalOutput")
    g = nc.dram_tensor("gtab", [GROWS, RW], BF16)

    with tile.TileContext(nc) as tc:
        with (
            tc.tile_pool(name="const", bufs=1) as cp,
            tc.tile_pool(name="proj", bufs=3) as pp,
            tc.tile_pool(name="projps", bufs=4, space="PSUM") as ppp,
            tc.tile_pool(name="agg", bufs=2) as ag,
            tc.tile_pool(name="small", bufs=3) as sm,
            tc.tile_pool(name="accps", bufs=2, space="PSUM") as psp,
        ):
            # resident constants
            w_sb = cp.tile([DIN, WC], BF16)
            nc.sync.dma_start(w_sb[:], wmat[:])
            idxs_res = cp.tile([128, S8], I16)
            nc.sync.dma_start(idxs_res[:], idxsd[:])
            iota_r = cp.tile([P, P], I32)
            nc.gpsimd.iota(iota_r[:], pattern=[[1, P]], base=0, channel_multiplier=0)
            iota_rf = cp.tile([P, P], F32)
            nc.vector.tensor_copy(iota_rf[:], iota_r[:])
            iota_p = cp.tile([P, 1], I32)
            nc.gpsimd.iota(iota_p[:], pattern=[[1, 1]], base=0, channel_multiplier=1)
            iota_pf = cp.tile([P, 1], F32)
            nc.vector.tensor_copy(iota_pf[:], iota_p[:])
            ident = cp.tile([P, P], BF16)
            nc.vector.tensor_scalar(
                out=ident[:],
                in0=iota_rf[:],
                scalar1=iota_pf[:, 0:1],
                scalar2=None,
                op0=OP.is_equal,
            )
            # pad row: ft/er = 0, el = PAD_EL -> exp(leaky(lg)) == 0
            padt = cp.tile([1, RW], BF16)
            nc.gpsimd.memset(padt[:], 0.0)
            nc.gpsimd.memset(padt[:, 256:260], PAD_EL)
            nc.gpsimd.memset(padt[:, WC + 256 : WC + 260], PAD_EL)
            nc.sync.dma_start(out=g[NPOS : NPOS + 1, :], in_=padt[:])

            # ---- projection: g[q] = [ft|el|er](b0) | [ft|el|er](b1) ----
            hw = (nc.sync, nc.scalar)
            for b in range(B if stage in ('all', 'proj') else 0):
                for blk in range(NBLK):
                    j = b * NBLK + blk
                    ftb = pp.tile([DIN, QBLK], BF16, tag="ftb")
                    hw[j % 2].dma_start(
                        ftb[:],
                        featT[:, b * NPOS + blk * QBLK : b * NPOS + (blk + 1) * QBLK],
                    )
                    pw = pp.tile([P, 8, WC], BF16, tag="pw")
                    for i in range(8):
                        po = ppp.tile([P, WC], F32, tag="po")
                        nc.tensor.matmul(
                            po[:],
                            lhsT=ftb[:, i * P : (i + 1) * P],
                            rhs=w_sb[:],
                            start=True,
                            stop=True,
                        )
                        if i % 2 == 0:
                            nc.vector.tensor_copy(pw[:, i, :], po[:])
                        else:
                            nc.scalar.activation(pw[:, i, :], po[:], AF.Copy)
                    dst_ap = g[
                        blk * QBLK : (blk + 1) * QBLK, b * WC : (b + 1) * WC
                    ].rearrange("(i p) c -> p i c", p=P)
                    hw[(j + 1) % 2].dma_start(out=dst_ap, in_=pw[:])

            # ---- aggregation: per tile, slot-aligned segment softmax+sum ----
            off8 = 0
            for t in range(NT if stage in ('all', 'agg') else 0):
                Kt = K[t]
                KP = Kt + 1
                gt = ag.tile([P, KP1M, RW], BF16, tag="gt")
                # SWDGE descriptor ring holds ~1024 pairs: cap 8 chunks/call
                for g0 in range(0, KP, 8):
                    gn = min(8, KP - g0)
                    nc.gpsimd.dma_gather(
                        out_ap=gt[:, g0 : g0 + gn, :],
                        in_ap=g[:],
                        idxs_ap=idxs_res[:, off8 + g0 * 8 : off8 + (g0 + gn) * 8],
                        num_idxs=gn * P,
                        num_idxs_reg=gn * P,
                        elem_size=RW,
                    )
                off8 += KP * 8

                # logits lg[p, (b,h), k] = el_src + er_own
                lg = sm.tile([P, B * H, KP1M], BF16, tag="lg")
                elv = gt[:, 0:Kt, 0 : 2 * WC].rearrange(
                    "p k (b r) -> p b r k", r=WC
                )[:, :, HD : HD + H, :]
                erv = gt[:, Kt, 0 : 2 * WC].rearrange("p (b r) -> p b r", r=WC)[
                    :, :, HD + H : HD + 2 * H
                ]
                lgv = lg[:].rearrange("p (b h) k -> p b h k", h=H)[:, :, :, 0:Kt]
                nc.vector.tensor_tensor(
                    out=lgv,
                    in0=elv,
                    in1=erv[:, :, :, None].to_broadcast([P, B, H, Kt]),
                    op=OP.add,
                )
                l1 = sm.tile([P, B * H, KP1M], BF16, tag="l1")
                nc.vector.scalar_tensor_tensor(
                    out=l1[:, :, 0:Kt],
                    in0=lg[:, :, 0:Kt],
                    scalar=NEG,
                    in1=lg[:, :, 0:Kt],
                    op0=OP.mult,
                    op1=OP.max,
                )
                ex = sm.tile([P, B * H, KP1M], BF16, tag="ex")
                nc.scalar.activation(ex[:, :, 0:Kt], l1[:, :, 0:Kt], AF.Exp)

                # messages: ft *= ex (in-place, per batch)
                for b in range(B):
                    ftv = gt[:, 0:Kt, b * WC : b * WC + HD].rearrange(
                        "p k (h d) -> p k h d", d=D
                    )
                    exv = ex[:].rearrange("p (b h) k -> p b h k", h=H)[
                        :, b, :, 0:Kt
                    ].rearrange("p h k -> p k h")
                    nc.vector.tensor_tensor(
                        out=ftv,
                        in0=ftv,
                        in1=exv[:, :, :, None].to_broadcast([P, Kt, H, D]),
                        op=OP.mult,
                    )

                # segment sum via identity matmuls accumulating in PSUM
                acc = psp.tile([P, FT], F32, tag="acc")
                for k in range(Kt):
                    rhs = gt[:, k, 0 : 2 * WC].rearrange("p (b r) -> p b r", r=WC)[
                        :, :, 0:HD
                    ]
                    nc.tensor.matmul(
                        acc[:],
                        lhsT=ident[:],
                        rhs=rhs,
                        start=(k == 0),
                        stop=(k == Kt - 1),
                    )

                dsum = sm.tile([P, B * H], F32, tag="dsum")
                nc.vector.tensor_reduce(
                    out=dsum[:],
                    in_=ex[:, :, 0:Kt],
                    axis=mybir.AxisListType.X,
                    op=OP.add,
                )
                dse = sm.tile([P, B * H], F32, tag="dse")
                nc.vector.tensor_scalar_add(dse[:], dsum[:], 1e-20)
                rcp = sm.tile([P, B * H, 1], F32, tag="rcp")
                nc.vector.reciprocal(rcp[:, :, 0], dse[:])
                o1 = sm.tile([P, B * H, D], F32, tag="o1")
                nc.vector.tensor_tensor(
                    out=o1[:],
                    in0=acc[:].rearrange("p (q d) -> p q d", d=D),
                    in1=rcp[:].to_broadcast([P, B * H, D]),
                    op=OP.mult,
                )
                og = sm.tile([P, FT], F32, tag="og")
                nc.vector.scalar_tensor_tensor(
                    out=og[:],
                    in0=o1[:].rearrange("p q d -> p (q d)"),
                    scalar=NEG,
                    in1=o1[:].rearrange("p q d -> p (q d)"),
                    op0=OP.mult,
                    op1=OP.max,
                )
                nc.sync.dma_start(out=outd[t * P : (t + 1) * P, :], in_=og[:])

    nc.compile()
    return nc


def _make_inputs(feat, W, attn_l, attn_r, src, dst, n_nodes=N, n_cores=NCORES):
    feat = np.asarray(feat, dtype=np.float32)
    W = np.asarray(W, dtype=np.float32)
    attn_l = np.asarray(attn_l, dtype=np.float32)
    attn_r = np.asarray(attn_r, dtype=np.float32)
    src = np.asarray(src)
    dst = np.asarray(dst)

    K, idxs_all, pos2node = _host_prep(src, dst)

    real = pos2node >= 0
    ftp = np.zeros((B, NPOS, DIN), np.float32)
    ftp[:, real, :] = feat[pos2node[real]].transpose(1, 0, 2)
    featT = np.ascontiguousarray(ftp.reshape(B * NPOS, DIN).T).astype(
        ml_dtypes.bfloat16
    )
    Wl = (W.reshape(DIN, H, D) * attn_l[None]).sum(-1)
    Wr = (W.reshape(DIN, H, D) * attn_r[None]).sum(-1)
    wmat = np.concatenate([W, Wl, Wr], axis=1).astype(ml_dtypes.bfloat16)

    in_maps = [
        {"featT": featT, "wmat": wmat, "idxs": idxs_all[c]} for c in range(n_cores)
    ]
    return K, in_maps, pos2node


_CACHE = {}


def kernel(feat, W, attn_l, attn_r, src, dst):
    K, in_maps, pos2node = _make_inputs(feat, W, attn_l, attn_r, src, dst)
    key = tuple(K)
    if key not in _CACHE:
        _CACHE[key] = _build(K)
    nc = _CACHE[key]
    res = run_bass_kernel_spmd(nc, in_maps, list(range(NCORES))).results
    out = np.empty((N, B, H, D), np.float32)
    for c in range(NCORES):
        nodes = pos2node[c * NPOSC : c * NPOSC + NPC]
        out[nodes] = res[c]["out"][:NPC].reshape(NPC, B, H, D)
    return out


if __name__ == "__main__":
    rng = np.random.default_rng(0)
    feat = rng.standard_normal((N, B, DIN), dtype=np.float32)
    W = rng.standard_normal((DIN, H * D), dtype=np.float32) / np.sqrt(DIN)
    al = rng.standard_normal((H, D), dtype=np.float32) * 0.1
    ar = rng.standard_normal((H, D), dtype=np.float32) * 0.1
    src = rng.integers(0, N, E).astype(np.int32)
    dst = rng.integers(0, N, E).astype(np.int32)
    out = kernel(feat=feat, W=W, attn_l=al, attn_r=ar, src=src, dst=dst)
    print(out.shape, out.dtype, np.abs(out).mean())


# revision 7
# speedup vs baseline: 1.8667x; 1.3500x over previous
"""Trainium2 Bass kernel for BatchGATConv (GAT message passing).

Strategy (8 NeuronCores, SPMD, dst-partitioned):
  - Host: in-degree-sort each core's 2500 dst nodes into 20 tiles of 128 so
    every tile has near-uniform degree; chunk k of a tile holds the k-th
    in-edge of each of the tile's 128 nodes (slot-aligned, so dst-local ==
    partition and no one-hot matmuls are needed). Padded slots point at a
    dedicated pad row whose attention logit is -150 (=> exp == 0).
  - Projection (replicated on all cores, bf16): g[pos] row pair
    [ft(b0)|el|er | ft(b1)|el|er | pad] with attn_l/attn_r folded into the
    weight matrix on the host; 1024-row blocks, batched DMAs.
  - Aggregation per tile: ONE dma_gather fetches all (K+1)*128 row pairs
    (the +1 chunk gathers the tile's own rows for er_dst); logits
    lg = el_src + er_dst, leaky, exp on ACT; messages ft*ex scaled in-place
    on DVE; PSUM-accumulated via identity matmuls (per-partition segment sum);
    denominator via DVE reduce; out = leaky(num/den).
"""

import numpy as np

try:
    import concourse.bass as bass
except ImportError:  # pragma: no cover
    import sys

    sys.path.insert(0, "/opt/trn_rl_repo")
    import concourse.bass as bass

import ml_dtypes
import concourse.bacc as bacc
import concourse.mybir as mybir
import concourse.tile as tile
from concourse.bass_utils import run_bass_kernel_spmd

P = 128
F32 = mybir.dt.float32
BF16 = mybir.dt.bfloat16
I32 = mybir.dt.int32
I16 = mybir.dt.int16

# problem constants
N, B, DIN, H, D, E = 20000, 2, 128, 4, 64, 320000
NEG = 0.2
NCORES = 8
NPC = N // NCORES  # 2500 dst nodes per core
NT = 20  # node tiles per core
NPOSC = NT * P  # 2560 positions per core (60 pad positions)
NPOS = NCORES * NPOSC  # 20480 global positions
PADROW = NPOS  # pad pair row index
GROWS = NPOS + 2
WC = 264  # per-(n,b) block: [ft 256 | el 4 | er 4]
HD = H * D  # 256
FT = B * HD  # 512
RW = 640  # g row width (2*WC + 112 pad; 1280B, %256 for dma_gather)
PAD_EL = -150.0
QBLK = 1024  # projection rows per block
NBLK = NPOS // QBLK  # 20


def _host_prep(src, dst):
    """Degree-sort nodes per core; build per-core wrapped int16 gather tables.

    Returns (K, idxs_all, pos2node):
      K: per-tile chunk count, shared across cores (max in-degree in tile)
      idxs_all[c]: [16, sum((K+1)*8)] int16 wrapped gather indices
      pos2node: [NPOS] original node id per position (-1 = pad)
    """
    deg = np.bincount(dst, minlength=N).astype(np.int64)
    pos2node = np.full(NPOS, -1, np.int64)
    for c in range(NCORES):
        ids = np.arange(c * NPC, (c + 1) * NPC)
        order = np.argsort(-deg[ids], kind="stable")
        pos2node[c * NPOSC : c * NPOSC + NPC] = ids[order]
    node2pos = np.empty(N, np.int64)
    real = pos2node >= 0
    node2pos[pos2node[real]] = np.nonzero(real)[0]

    first = np.arange(NCORES)[:, None] * NPOSC + np.arange(NT)[None, :] * P
    K = np.maximum(deg[pos2node[first]].max(axis=0), 1)  # [NT]

    dpos = node2pos[dst]
    spos = node2pos[src]
    order_e = np.argsort(dpos, kind="stable")
    ds = dpos[order_e]
    ss = spos[order_e]
    starts = np.searchsorted(ds, np.arange(NPOS + 1))
    k_e = np.arange(E) - starts[ds]

    idxs_all = []
    for c in range(NCORES):
        cols = []
        for t in range(NT):
            Kt = int(K[t])
            base = c * NPOSC + t * P
            sl = np.full((Kt + 1) * P, PADROW, np.int64)
            e0, e1 = starts[base], starts[base + P]
            pp_ = ds[e0:e1] - base
            sl[k_e[e0:e1] * P + pp_] = ss[e0:e1]
            sl[Kt * P : (Kt + 1) * P] = base + np.arange(P)
            cols.append(sl.reshape(-1, 16).T)
        idxs_all.append(
            np.ascontiguousarray(
                np.tile(np.concatenate(cols, axis=1), (8, 1))
            ).astype(np.int16)
        )
    return list(map(int, K)), idxs_all, pos2node


def _build(K, stage='all'):
    S8 = sum((k + 1) * 8 for k in K)
    KP1M = max(K) + 1
    AF = mybir.ActivationFunctionType
    OP = mybir.AluOpType

    nc = bacc.Bacc(trn_type="TRN2", num_swdge_queues=2)
    featT = nc.dram_tensor("featT", [DIN, B * NPOS], BF16, kind="ExternalInput")
    wmat = nc.dram_tensor("wmat", [DIN, WC], BF16, kind="ExternalInput")
    idxsd = nc.dram_tensor("idxs", [128, S8], I16, kind="ExternalInput")
    outd = nc.dram_tensor("out", [NPOSC, FT], F32, kind="ExternalOutput")
    g = nc.dram_tensor("gtab", [GROWS, RW], BF16)

    with tile.TileContext(nc) as tc:
        with (
            tc.tile_pool(name="const", bufs=1) as cp,
            tc.tile_pool(name="proj", bufs=3) as pp,
            tc.tile_pool(name="projps", bufs=4, space="PSUM") as ppp,
            tc.tile_pool(name="agg", bufs=3) as ag,
            tc.tile_pool(name="small", bufs=3) as sm,
            tc.tile_pool(name="accps", bufs=2, space="PSUM") as psp,
        ):
            # resident constants
            w_sb = cp.tile([DIN, WC], BF16)
            nc.sync.dma_start(w_sb[:], wmat[:])
            idxs_res = cp.tile([128, S8], I16)
            nc.sync.dma_start(idxs_res[:], idxsd[:])
            iota_r = cp.tile([P, P], I32)
            nc.gpsimd.iota(iota_r[:], pattern=[[1, P]], base=0, channel_multiplier=0)
            iota_rf = cp.tile([P, P], F32)
            nc.vector.tensor_copy(iota_rf[:], iota_r[:])
            iota_p = cp.tile([P, 1], I32)
            nc.gpsimd.iota(iota_p[:], pattern=[[1, 1]], base=0, channel_multiplier=1)
            iota_pf = cp.tile([P, 1], F32)
            nc.vector.tensor_copy(iota_pf[:], iota_p[:])
            ident = cp.tile([P, P], BF16)
            nc.vector.tensor_scalar(
                out=ident[:],
                in0=iota_rf[:],
                scalar1=iota_pf[:, 0:1],
                scalar2=None,
                op0=OP.is_equal,
            )
            # pad row: ft/er = 0, el = PAD_EL -> exp(leaky(lg)) == 0
            padt = cp.tile([1, RW], BF16)
            nc.gpsimd.memset(padt[:], 0.0)
            nc.gpsimd.memset(padt[:, 256:260], PAD_EL)
            nc.gpsimd.memset(padt[:, WC + 256 : WC + 260], PAD_EL)
            nc.sync.dma_start(out=g[NPOS : NPOS + 1, :], in_=padt[:])

            # ---- projection: g[q] = [ft|el|er](b0) | [ft|el|er](b1) ----
            hw = (nc.sync, nc.scalar)
            for b in range(B if stage in ('all', 'proj') else 0):
                for blk in range(NBLK):
                    j = b * NBLK + blk
                    ftb = pp.tile([DIN, QBLK], BF16, tag="ftb")
                    hw[j % 2].dma_start(
                        ftb[:],
                        featT[:, b * NPOS + blk * QBLK : b * NPOS + (blk + 1) * QBLK],
                    )
                    pw = pp.tile([P, 8, WC], BF16, tag="pw")
                    for i in range(8):
                        po = ppp.tile([P, WC], F32, tag="po")
                        nc.tensor.matmul(
                            po[:],
                            lhsT=ftb[:, i * P : (i + 1) * P],
                            rhs=w_sb[:],
                            start=True,
                            stop=True,
                        )
                        if i % 2 == 0:
                            nc.vector.tensor_copy(pw[:, i, :], po[:])
                        else:
                            nc.scalar.activation(pw[:, i, :], po[:], AF.Copy)
                    dst_ap = g[
                        blk * QBLK : (blk + 1) * QBLK, b * WC : (b + 1) * WC
                    ].rearrange("(i p) c -> p i c", p=P)
                    hw[(j + 1) % 2].dma_start(out=dst_ap, in_=pw[:])

            # ---- aggregation: per tile, slot-aligned segment softmax+sum ----
            off8 = 0
            for t in range(NT if stage in ('all', 'agg') else 0):
                Kt = K[t]
                KP = Kt + 1
                gt = ag.tile([P, KP1M, RW], BF16, tag="gt")
                # SWDGE descriptor ring holds ~1024 pairs: cap 8 chunks/call;
                # alternate the two SWDGE queues to overlap Q7 descriptor gen
                for g0 in range(0, KP, 8):
                    gn = min(8, KP - g0)
                    nc.gpsimd.dma_gather(
                        out_ap=gt[:, g0 : g0 + gn, :],
                        in_ap=g[:],
                        idxs_ap=idxs_res[:, off8 + g0 * 8 : off8 + (g0 + gn) * 8],
                        num_idxs=gn * P,
                        num_idxs_reg=gn * P,
                        elem_size=RW,
                        queue_num=(t * 7 + g0 // 8) % 2,
                    )
                off8 += KP * 8

                # logits lg[p, (b,h), k] = el_src + er_own
                lg = sm.tile([P, B * H, KP1M], BF16, tag="lg")
                elv = gt[:, 0:Kt, 0 : 2 * WC].rearrange(
                    "p k (b r) -> p b r k", r=WC
                )[:, :, HD : HD + H, :]
                erv = gt[:, Kt, 0 : 2 * WC].rearrange("p (b r) -> p b r", r=WC)[
                    :, :, HD + H : HD + 2 * H
                ]
                lgv = lg[:].rearrange("p (b h) k -> p b h k", h=H)[:, :, :, 0:Kt]
                nc.vector.tensor_tensor(
                    out=lgv,
                    in0=elv,
                    in1=erv[:, :, :, None].to_broadcast([P, B, H, Kt]),
                    op=OP.add,
                )
                l1 = sm.tile([P, B * H, KP1M], BF16, tag="l1")
                nc.vector.scalar_tensor_tensor(
                    out=l1[:, :, 0:Kt],
                    in0=lg[:, :, 0:Kt],
                    scalar=NEG,
                    in1=lg[:, :, 0:Kt],
                    op0=OP.mult,
                    op1=OP.max,
                )
                ex = sm.tile([P, B * H, KP1M], BF16, tag="ex")
                nc.scalar.activation(ex[:, :, 0:Kt], l1[:, :, 0:Kt], AF.Exp)

                # messages: ft *= ex (in-place, per batch)
                for b in range(B):
                    ftv = gt[:, 0:Kt, b * WC : b * WC + HD].rearrange(
                        "p k (h d) -> p k h d", d=D
                    )
                    exv = ex[:].rearrange("p (b h) k -> p b h k", h=H)[
                        :, b, :, 0:Kt
                    ].rearrange("p h k -> p k h")
                    nc.vector.tensor_tensor(
                        out=ftv,
                        in0=ftv,
                        in1=exv[:, :, :, None].to_broadcast([P, Kt, H, D]),
                        op=OP.mult,
                    )

                # segment sum via identity matmuls accumulating in PSUM
                acc = psp.tile([P, FT], F32, tag="acc")
                for k in range(Kt):
                    rhs = gt[:, k, 0 : 2 * WC].rearrange("p (b r) -> p b r", r=WC)[
                        :, :, 0:HD
                    ]
                    nc.tensor.matmul(
                        acc[:],
                        lhsT=ident[:],
                        rhs=rhs,
                        start=(k == 0),
                        stop=(k == Kt - 1),
                    )

                dsum = sm.tile([P, B * H], F32, tag="dsum")
                nc.vector.tensor_reduce(
                    out=dsum[:],
                    in_=ex[:, :, 0:Kt],
                    axis=mybir.AxisListType.X,
                    op=OP.add,
                )
                dse = sm.tile([P, B * H], F32, tag="dse")
                nc.vector.tensor_scalar_add(dse[:], dsum[:], 1e-20)
                rcp = sm.tile([P, B * H, 1], F32, tag="rcp")
                nc.vector.reciprocal(rcp[:, :, 0], dse[:])
                o1 = sm.tile([P, B * H, D], F32, tag="o1")
                nc.vector.tensor_tensor(
                    out=o1[:],
                    in0=acc[:].rearrange("p (q d) -> p q d", d=D),
                    in1=rcp[:].to_broadcast([P, B * H, D]),
                    op=OP.mult,
                )
                og = sm.tile([P, FT], F32, tag="og")
                nc.vector.scalar_tensor_tensor(
                    out=og[:],
                    in0=o1[:].rearrange("p q d -> p (q d)"),
                    scalar=NEG,
                    in1=o1[:].rearrange("p q d -> p (q d)"),
                    op0=OP.mult,
                    op1=OP.max,
                )
                nc.sync.dma_start(out=outd[t * P : (t + 1) * P, :], in_=og[:])

    nc.compile()
    return nc


def _make_inputs(feat, W, attn_l, attn_r, src, dst, n_nodes=N, n_cores=NCORES):
    feat = np.asarray(feat, dtype=np.float32)
    W = np.asarray(W, dtype=np.float32)
    attn_l = np.asarray(attn_l, dtype=np.float32)
    attn_r = np.asarray(attn_r, dtype=np.float32)
    src = np.asarray(src)
    dst = np.asarray(dst)

    K, idxs_all, pos2node = _host_prep(src, dst)

    real = pos2node >= 0
    ftp = np.zeros((B, NPOS, DIN), np.float32)
    ftp[:, real, :] = feat[pos2node[real]].transpose(1, 0, 2)
    featT = np.ascontiguousarray(ftp.reshape(B * NPOS, DIN).T).astype(
        ml_dtypes.bfloat16
    )
    Wl = (W.reshape(DIN, H, D) * attn_l[None]).sum(-1)
    Wr = (W.reshape(DIN, H, D) * attn_r[None]).sum(-1)
    wmat = np.concatenate([W, Wl, Wr], axis=1).astype(ml_dtypes.bfloat16)

    in_maps = [
        {"featT": featT, "wmat": wmat, "idxs": idxs_all[c]} for c in range(n_cores)
    ]
    return K, in_maps, pos2node


_CACHE = {}


def kernel(feat, W, attn_l, attn_r, src, dst):
    K, in_maps, pos2node = _make_inputs(feat, W, attn_l, attn_r, src, dst)
    key = tuple(K)
    if key not in _CACHE:
        _CACHE[key] = _build(K)
    nc = _CACHE[key]
    res = run_bass_kernel_spmd(nc, in_maps, list(range(NCORES))).results
    out = np.empty((N, B, H, D), np.float32)
    for c in range(NCORES):
        nodes = pos2node[c * NPOSC : c * NPOSC + NPC]
        out[nodes] = res[c]["out"][:NPC].reshape(NPC, B, H, D)
    return out


if __name__ == "__main__":
    rng = np.random.default_rng(0)
    feat = rng.standard_normal((N, B, DIN), dtype=np.float32)
    W = rng.standard_normal((DIN, H * D), dtype=np.float32) / np.sqrt(DIN)
    al = rng.standard_normal((H, D), dtype=np.float32) * 0.1
    ar = rng.standard_normal((H, D), dtype=np.float32) * 0.1
    src = rng.integers(0, N, E).astype(np.int32)
    dst = rng.integers(0, N, E).astype(np.int32)
    out = kernel(feat=feat, W=W, attn_l=al, attn_r=ar, src=src, dst=dst)
    print(out.shape, out.dtype, np.abs(out).mean())


# revision 10
# speedup vs baseline: 1.8985x; 1.0170x over previous
"""Trainium2 Bass kernel for BatchGATConv (GAT message passing).

Strategy (8 NeuronCores, SPMD, dst-partitioned):
  - Host: in-degree-sort each core's 2500 dst nodes into 20 tiles of 128 so
    every tile has near-uniform degree; chunk k of a tile holds the k-th
    in-edge of each of the tile's 128 nodes (slot-aligned, so dst-local ==
    partition and no one-hot matmuls are needed). Padded slots point at a
    dedicated pad row whose attention logit is -150 (=> exp == 0).
  - Projection (replicated on all cores, bf16): g[pos] row pair
    [ft(b0)|el|er | ft(b1)|el|er | pad] with attn_l/attn_r folded into the
    weight matrix on the host; 1024-row blocks, batched DMAs.
  - Aggregation per tile: ONE dma_gather fetches all (K+1)*128 row pairs
    (the +1 chunk gathers the tile's own rows for er_dst); logits
    lg = el_src + er_dst, leaky, exp on ACT; messages ft*ex scaled in-place
    on DVE; PSUM-accumulated via identity matmuls (per-partition segment sum);
    denominator via DVE reduce; out = leaky(num/den).
"""

import numpy as np

try:
    import concourse.bass as bass
except ImportError:  # pragma: no cover
    import sys

    sys.path.insert(0, "/opt/trn_rl_repo")
    import concourse.bass as bass

import ml_dtypes
import concourse.bacc as bacc
import concourse.mybir as mybir
import concourse.tile as tile
from concourse.bass_utils import run_bass_kernel_spmd

P = 128
F32 = mybir.dt.float32
BF16 = mybir.dt.bfloat16
I32 = mybir.dt.int32
I16 = mybir.dt.int16

# problem constants
N, B, DIN, H, D, E = 20000, 2, 128, 4, 64, 320000
NEG = 0.2
NCORES = 8
NPC = N // NCORES  # 2500 dst nodes per core
NT = 20  # node tiles per core
NPOSC = NT * P  # 2560 positions per core (60 pad positions)
NPOS = NCORES * NPOSC  # 20480 global positions
PADROW = NPOS  # pad pair row index
GROWS = NPOS + 2
WC = 264  # per-(n,b) block: [ft 256 | el 4 | er 4]
HD = H * D  # 256
FT = B * HD  # 512
RW = 640  # g row width (2*WC + 112 pad; 1280B, %256 for dma_gather)
PAD_EL = -150.0
QBLK = 1024  # projection rows per block
NBLK = NPOS // QBLK  # 20
NSWQ = 4  # SWDGE queues (Q7 descriptor-gen parallelism)


def _host_prep(src, dst):
    """Degree-sort nodes per core; build per-core wrapped int16 gather tables.

    Returns (K, idxs_all, pos2node):
      K: per-tile chunk count, shared across cores (max in-degree in tile)
      idxs_all[c]: [16, sum((K+1)*8)] int16 wrapped gather indices
      pos2node: [NPOS] original node id per position (-1 = pad)
    """
    deg = np.bincount(dst, minlength=N).astype(np.int64)
    pos2node = np.full(NPOS, -1, np.int64)
    for c in range(NCORES):
        ids = np.arange(c * NPC, (c + 1) * NPC)
        order = np.argsort(-deg[ids], kind="stable")
        pos2node[c * NPOSC : c * NPOSC + NPC] = ids[order]
    node2pos = np.empty(N, np.int64)
    real = pos2node >= 0
    node2pos[pos2node[real]] = np.nonzero(real)[0]

    first = np.arange(NCORES)[:, None] * NPOSC + np.arange(NT)[None, :] * P
    K = np.maximum(deg[pos2node[first]].max(axis=0), 1)  # [NT]

    dpos = node2pos[dst]
    spos = node2pos[src]
    order_e = np.argsort(dpos, kind="stable")
    ds = dpos[order_e]
    ss = spos[order_e]
    starts = np.searchsorted(ds, np.arange(NPOS + 1))
    k_e = np.arange(E) - starts[ds]

    idxs_all = []
    for c in range(NCORES):
        cols = []
        for t in range(NT):
            Kt = int(K[t])
            base = c * NPOSC + t * P
            sl = np.full((Kt + 1) * P, PADROW, np.int64)
            e0, e1 = starts[base], starts[base + P]
            pp_ = ds[e0:e1] - base
            sl[k_e[e0:e1] * P + pp_] = ss[e0:e1]
            sl[Kt * P : (Kt + 1) * P] = base + np.arange(P)
            cols.append(sl.reshape(-1, 16).T)
        idxs_all.append(
            np.ascontiguousarray(
                np.tile(np.concatenate(cols, axis=1), (8, 1))
            ).astype(np.int16)
        )
    return list(map(int, K)), idxs_all, pos2node


def _build(K, stage='all'):
    S8 = sum((k + 1) * 8 for k in K)
    KP1M = max(K) + 1
    AF = mybir.ActivationFunctionType
    OP = mybir.AluOpType

    nc = bacc.Bacc(trn_type="TRN2", num_swdge_queues=NSWQ)
    featT = nc.dram_tensor("featT", [DIN, B * NPOS], BF16, kind="ExternalInput")
    wmat = nc.dram_tensor("wmat", [DIN, WC], BF16, kind="ExternalInput")
    idxsd = nc.dram_tensor("idxs", [128, S8], I16, kind="ExternalInput")
    outd = nc.dram_tensor("out", [NPOSC, FT], F32, kind="ExternalOutput")
    g = nc.dram_tensor("gtab", [GROWS, RW], BF16)

    with tile.TileContext(nc) as tc:
        with (
            tc.tile_pool(name="const", bufs=1) as cp,
            tc.tile_pool(name="proj", bufs=3) as pp,
            tc.tile_pool(name="projps", bufs=4, space="PSUM") as ppp,
            tc.tile_pool(name="agg", bufs=3) as ag,
            tc.tile_pool(name="small", bufs=3) as sm,
            tc.tile_pool(name="accps", bufs=2, space="PSUM") as psp,
        ):
            # resident constants
            w_sb = cp.tile([DIN, WC], BF16)
            nc.sync.dma_start(w_sb[:], wmat[:])
            idxs_res = cp.tile([128, S8], I16)
            nc.sync.dma_start(idxs_res[:], idxsd[:])
            iota_r = cp.tile([P, P], I32)
            nc.gpsimd.iota(iota_r[:], pattern=[[1, P]], base=0, channel_multiplier=0)
            iota_rf = cp.tile([P, P], F32)
            nc.vector.tensor_copy(iota_rf[:], iota_r[:])
            iota_p = cp.tile([P, 1], I32)
            nc.gpsimd.iota(iota_p[:], pattern=[[1, 1]], base=0, channel_multiplier=1)
            iota_pf = cp.tile([P, 1], F32)
            nc.vector.tensor_copy(iota_pf[:], iota_p[:])
            ident = cp.tile([P, P], BF16)
            nc.vector.tensor_scalar(
                out=ident[:],
                in0=iota_rf[:],
                scalar1=iota_pf[:, 0:1],
                scalar2=None,
                op0=OP.is_equal,
            )
            # pad row: ft/er = 0, el = PAD_EL -> exp(leaky(lg)) == 0
            padt = cp.tile([1, RW], BF16)
            nc.gpsimd.memset(padt[:], 0.0)
            nc.gpsimd.memset(padt[:, 256:260], PAD_EL)
            nc.gpsimd.memset(padt[:, WC + 256 : WC + 260], PAD_EL)
            nc.sync.dma_start(out=g[NPOS : NPOS + 1, :], in_=padt[:])

            # ---- projection: g[q] = [ft|el|er](b0) | [ft|el|er](b1) ----
            hw = (nc.sync, nc.scalar)
            for b in range(B if stage in ('all', 'proj') else 0):
                for blk in range(NBLK):
                    j = b * NBLK + blk
                    ftb = pp.tile([DIN, QBLK], BF16, tag="ftb")
                    hw[j % 2].dma_start(
                        ftb[:],
                        featT[:, b * NPOS + blk * QBLK : b * NPOS + (blk + 1) * QBLK],
                    )
                    pw = pp.tile([P, 8, WC], BF16, tag="pw")
                    for i in range(8):
                        po = ppp.tile([P, WC], F32, tag="po")
                        nc.tensor.matmul(
                            po[:],
                            lhsT=ftb[:, i * P : (i + 1) * P],
                            rhs=w_sb[:],
                            start=True,
                            stop=True,
                        )
                        if i % 2 == 0:
                            nc.vector.tensor_copy(pw[:, i, :], po[:])
                        else:
                            nc.scalar.activation(pw[:, i, :], po[:], AF.Copy)
                    dst_ap = g[
                        blk * QBLK : (blk + 1) * QBLK, b * WC : (b + 1) * WC
                    ].rearrange("(i p) c -> p i c", p=P)
                    hw[(j + 1) % 2].dma_start(out=dst_ap, in_=pw[:])

            # ---- aggregation: per tile, slot-aligned segment softmax+sum ----
            off8 = 0
            ncall = 0
            for t in range(NT if stage in ('all', 'agg') else 0):
                Kt = K[t]
                KP = Kt + 1
                gt = ag.tile([P, KP1M, RW], BF16, tag="gt")
                # SWDGE descriptor ring holds ~1024 pairs: cap 8 chunks/call;
                # alternate the two SWDGE queues to overlap Q7 descriptor gen
                for g0 in range(0, KP, 8):
                    gn = min(8, KP - g0)
                    nc.gpsimd.dma_gather(
                        out_ap=gt[:, g0 : g0 + gn, :],
                        in_ap=g[:],
                        idxs_ap=idxs_res[:, off8 + g0 * 8 : off8 + (g0 + gn) * 8],
                        num_idxs=gn * P,
                        num_idxs_reg=gn * P,
                        elem_size=RW,
                        queue_num=ncall % NSWQ,
                    )
                    ncall += 1
                off8 += KP * 8

                # logits lg[p, k, (b,h)] = el_src + er_own (k-major: all the
                # downstream DVE/ACT ops stream contiguously)
                lg = sm.tile([P, KP1M, B * H], BF16, tag="lg")
                elv = gt[:, 0:Kt, 0 : 2 * WC].rearrange(
                    "p k (b r) -> p k b r", r=WC
                )[:, :, :, HD : HD + H]
                erv = gt[:, Kt, 0 : 2 * WC].rearrange("p (b r) -> p b r", r=WC)[
                    :, :, HD + H : HD + 2 * H
                ]
                lgv = lg[:, 0:Kt, :].rearrange("p k (b h) -> p k b h", h=H)
                nc.vector.tensor_tensor(
                    out=lgv,
                    in0=elv,
                    in1=erv[:, None, :, :].to_broadcast([P, Kt, B, H]),
                    op=OP.add,
                )
                l1 = sm.tile([P, KP1M, B * H], BF16, tag="l1")
                nc.vector.scalar_tensor_tensor(
                    out=l1[:, 0:Kt, :],
                    in0=lg[:, 0:Kt, :],
                    scalar=NEG,
                    in1=lg[:, 0:Kt, :],
                    op0=OP.mult,
                    op1=OP.max,
                )
                ex = sm.tile([P, KP1M, B * H], BF16, tag="ex")
                nc.scalar.activation(ex[:, 0:Kt, :], l1[:, 0:Kt, :], AF.Exp)

                # messages: ft *= ex (in-place, per batch)
                for b in range(B):
                    ftv = gt[:, 0:Kt, b * WC : b * WC + HD].rearrange(
                        "p k (h d) -> p k h d", d=D
                    )
                    exv = ex[:, 0:Kt, b * H : (b + 1) * H]
                    nc.vector.tensor_tensor(
                        out=ftv,
                        in0=ftv,
                        in1=exv[:, :, :, None].to_broadcast([P, Kt, H, D]),
                        op=OP.mult,
                    )

                # segment sum via identity matmuls accumulating in PSUM
                acc = psp.tile([P, FT], F32, tag="acc")
                for k in range(Kt):
                    rhs = gt[:, k, 0 : 2 * WC].rearrange("p (b r) -> p b r", r=WC)[
                        :, :, 0:HD
                    ]
                    nc.tensor.matmul(
                        acc[:],
                        lhsT=ident[:],
                        rhs=rhs,
                        start=(k == 0),
                        stop=(k == Kt - 1),
                    )

                dsum = sm.tile([P, B * H], F32, tag="dsum")
                nc.vector.tensor_reduce(
                    out=dsum[:],
                    in_=ex[:, 0:Kt, :].rearrange("p k q -> p q k"),
                    axis=mybir.AxisListType.X,
                    op=OP.add,
                )
                dse = sm.tile([P, B * H], F32, tag="dse")
                nc.vector.tensor_scalar_add(dse[:], dsum[:], 1e-20)
                rcp = sm.tile([P, B * H, 1], F32, tag="rcp")
                nc.vector.reciprocal(rcp[:, :, 0], dse[:])
                o1 = sm.tile([P, B * H, D], F32, tag="o1")
                nc.vector.tensor_tensor(
                    out=o1[:],
                    in0=acc[:].rearrange("p (q d) -> p q d", d=D),
                    in1=rcp[:].to_broadcast([P, B * H, D]),
                    op=OP.mult,
                )
                og = sm.tile([P, FT], F32, tag="og")
                nc.vector.scalar_tensor_tensor(
                    out=og[:],
                    in0=o1[:].rearrange("p q d -> p (q d)"),
                    scalar=NEG,
                    in1=o1[:].rearrange("p q d -> p (q d)"),
                    op0=OP.mult,
                    op1=OP.max,
                )
                nc.sync.dma_start(out=outd[t * P : (t + 1) * P, :], in_=og[:])

    nc.compile()
    return nc


def _make_inputs(feat, W, attn_l, attn_r, src, dst, n_nodes=N, n_cores=NCORES):
    feat = np.asarray(feat, dtype=np.float32)
    W = np.asarray(W, dtype=np.float32)
    attn_l = np.asarray(attn_l, dtype=np.float32)
    attn_r = np.asarray(attn_r, dtype=np.float32)
    src = np.asarray(src)
    dst = np.asarray(dst)

    K, idxs_all, pos2node = _host_prep(src, dst)

    real = pos2node >= 0
    ftp = np.zeros((B, NPOS, DIN), np.float32)
    ftp[:, real, :] = feat[pos2node[real]].transpose(1, 0, 2)
    featT = np.ascontiguousarray(ftp.reshape(B * NPOS, DIN).T).astype(
        ml_dtypes.bfloat16
    )
    Wl = (W.reshape(DIN, H, D) * attn_l[None]).sum(-1)
    Wr = (W.reshape(DIN, H, D) * attn_r[None]).sum(-1)
    wmat = np.concatenate([W, Wl, Wr], axis=1).astype(ml_dtypes.bfloat16)

    in_maps = [
        {"featT": featT, "wmat": wmat, "idxs": idxs_all[c]} for c in range(n_cores)
    ]
    return K, in_maps, pos2node


_CACHE = {}


def kernel(feat, W, attn_l, attn_r, src, dst):
    K, in_maps, pos2node = _make_inputs(feat, W, attn_l, attn_r, src, dst)
    key = tuple(K)
    if key not in _CACHE:
        _CACHE[key] = _build(K)
    nc = _CACHE[key]
    res = run_bass_kernel_spmd(nc, in_maps, list(range(NCORES))).results
    out = np.empty((N, B, H, D), np.float32)
    for c in range(NCORES):
        nodes = pos2node[c * NPOSC : c * NPOSC + NPC]
        out[nodes] = res[c]["out"][:NPC].reshape(NPC, B, H, D)
    return out


if __name__ == "__main__":
    rng = np.random.default_rng(0)
    feat = rng.standard_normal((N, B, DIN), dtype=np.float32)
    W = rng.standard_normal((DIN, H * D), dtype=np.float32) / np.sqrt(DIN)
    al = rng.standard_normal((H, D), dtype=np.float32) * 0.1
    ar = rng.standard_normal((H, D), dtype=np.float32) * 0.1
    src = rng.integers(0, N, E).astype(np.int32)
    dst = rng.integers(0, N, E).astype(np.int32)
    out = kernel(feat=feat, W=W, attn_l=al, attn_r=ar, src=src, dst=dst)
    print(out.shape, out.dtype, np.abs(out).mean())


# revision 13
# speedup vs baseline: 1.9003x; 1.0010x over previous
"""Trainium2 Bass kernel for BatchGATConv (GAT message passing).

Strategy (8 NeuronCores, SPMD, dst-partitioned):
  - Host: in-degree-sort each core's 2500 dst nodes into 20 tiles of 128 so
    every tile has near-uniform degree; chunk k of a tile holds the k-th
    in-edge of each of the tile's 128 nodes (slot-aligned, so dst-local ==
    partition and no one-hot matmuls are needed). Padded slots point at a
    dedicated pad row whose attention logit is -150 (=> exp == 0).
  - Projection (replicated on all cores, bf16): g[pos] row pair
    [ft(b0)|el|er | ft(b1)|el|er | pad] with attn_l/attn_r folded into the
    weight matrix on the host; 1024-row blocks, batched DMAs.
  - Aggregation per tile: ONE dma_gather fetches all (K+1)*128 row pairs
    (the +1 chunk gathers the tile's own rows for er_dst); logits
    lg = el_src + er_dst, leaky, exp on ACT; messages ft*ex scaled in-place
    on DVE; PSUM-accumulated via identity matmuls (per-partition segment sum);
    denominator via DVE reduce; out = leaky(num/den).
"""

import numpy as np

try:
    import concourse.bass as bass
except ImportError:  # pragma: no cover
    import sys

    sys.path.insert(0, "/opt/trn_rl_repo")
    import concourse.bass as bass

import ml_dtypes
import concourse.bacc as bacc
import concourse.mybir as mybir
import concourse.tile as tile
from concourse.bass_utils import run_bass_kernel_spmd

P = 128
F32 = mybir.dt.float32
BF16 = mybir.dt.bfloat16
I32 = mybir.dt.int32
I16 = mybir.dt.int16

# problem constants
N, B, DIN, H, D, E = 20000, 2, 128, 4, 64, 320000
NEG = 0.2
NCORES = 8
NPC = N // NCORES  # 2500 dst nodes per core
NT = 20  # node tiles per core
NPOSC = NT * P  # 2560 positions per core (60 pad positions)
NPOS = NCORES * NPOSC  # 20480 global positions
PADROW = NPOS  # pad pair row index
GROWS = NPOS + 2
WC = 264  # per-(n,b) block: [ft 256 | el 4 | er 4]
HD = H * D  # 256
FT = B * HD  # 512
RW = 640  # g row width (2*WC + 112 pad; 1280B, %256 for dma_gather)
PAD_EL = -150.0
QBLK = 1024  # projection rows per block
NBLK = NPOS // QBLK  # 20
NSWQ = 4  # SWDGE queues (Q7 descriptor-gen parallelism)


def _host_prep(src, dst):
    """Degree-sort nodes per core; build per-core wrapped int16 gather tables.

    Returns (K, idxs_all, pos2node):
      K: per-tile chunk count, shared across cores (max in-degree in tile)
      idxs_all[c]: [16, sum((K+1)*8)] int16 wrapped gather indices
      pos2node: [NPOS] original node id per position (-1 = pad)
    """
    deg = np.bincount(dst, minlength=N).astype(np.int64)
    pos2node = np.full(NPOS, -1, np.int64)
    for c in range(NCORES):
        ids = np.arange(c * NPC, (c + 1) * NPC)
        order = np.argsort(-deg[ids], kind="stable")
        pos2node[c * NPOSC : c * NPOSC + NPC] = ids[order]
    node2pos = np.empty(N, np.int64)
    real = pos2node >= 0
    node2pos[pos2node[real]] = np.nonzero(real)[0]

    first = np.arange(NCORES)[:, None] * NPOSC + np.arange(NT)[None, :] * P
    K = np.maximum(deg[pos2node[first]].max(axis=0), 1)  # [NT]

    dpos = node2pos[dst]
    spos = node2pos[src]
    order_e = np.argsort(dpos, kind="stable")
    ds = dpos[order_e]
    ss = spos[order_e]
    starts = np.searchsorted(ds, np.arange(NPOS + 1))
    k_e = np.arange(E) - starts[ds]

    idxs_all = []
    for c in range(NCORES):
        cols = []
        for t in range(NT):
            Kt = int(K[t])
            base = c * NPOSC + t * P
            sl = np.full((Kt + 1) * P, PADROW, np.int64)
            e0, e1 = starts[base], starts[base + P]
            pp_ = ds[e0:e1] - base
            sl[k_e[e0:e1] * P + pp_] = ss[e0:e1]
            sl[Kt * P : (Kt + 1) * P] = base + np.arange(P)
            cols.append(sl.reshape(-1, 16).T)
        idxs_all.append(
            np.ascontiguousarray(
                np.tile(np.concatenate(cols, axis=1), (8, 1))
            ).astype(np.int16)
        )
    return list(map(int, K)), idxs_all, pos2node


def _build(K, stage='all', lrelu=False):
    S8 = sum((k + 1) * 8 for k in K)
    KP1M = max(K) + 1
    AF = mybir.ActivationFunctionType
    OP = mybir.AluOpType

    nc = bacc.Bacc(trn_type="TRN2", num_swdge_queues=NSWQ)
    featT = nc.dram_tensor("featT", [DIN, B * NPOS], BF16, kind="ExternalInput")
    wmat = nc.dram_tensor("wmat", [DIN, WC], BF16, kind="ExternalInput")
    idxsd = nc.dram_tensor("idxs", [128, S8], I16, kind="ExternalInput")
    outd = nc.dram_tensor("out", [NPOSC, FT], F32, kind="ExternalOutput")
    g = nc.dram_tensor("gtab", [GROWS, RW], BF16)

    with tile.TileContext(nc) as tc:
        with (
            tc.tile_pool(name="const", bufs=1) as cp,
            tc.tile_pool(name="proj", bufs=3) as pp,
            tc.tile_pool(name="projps", bufs=4, space="PSUM") as ppp,
            tc.tile_pool(name="agg", bufs=3) as ag,
            tc.tile_pool(name="small", bufs=3) as sm,
            tc.tile_pool(name="accps", bufs=2, space="PSUM") as psp,
        ):
            # resident constants
            w_sb = cp.tile([DIN, WC], BF16)
            nc.sync.dma_start(w_sb[:], wmat[:])
            idxs_res = cp.tile([128, S8], I16)
            nc.sync.dma_start(idxs_res[:], idxsd[:])
            iota_r = cp.tile([P, P], I32)
            nc.gpsimd.iota(iota_r[:], pattern=[[1, P]], base=0, channel_multiplier=0)
            iota_rf = cp.tile([P, P], F32)
            nc.vector.tensor_copy(iota_rf[:], iota_r[:])
            iota_p = cp.tile([P, 1], I32)
            nc.gpsimd.iota(iota_p[:], pattern=[[1, 1]], base=0, channel_multiplier=1)
            iota_pf = cp.tile([P, 1], F32)
            nc.vector.tensor_copy(iota_pf[:], iota_p[:])
            ident = cp.tile([P, P], BF16)
            nc.vector.tensor_scalar(
                out=ident[:],
                in0=iota_rf[:],
                scalar1=iota_pf[:, 0:1],
                scalar2=None,
                op0=OP.is_equal,
            )
            # pad row: ft/er = 0, el = PAD_EL -> exp(leaky(lg)) == 0
            padt = cp.tile([1, RW], BF16)
            nc.gpsimd.memset(padt[:], 0.0)
            nc.gpsimd.memset(padt[:, 256:260], PAD_EL)
            nc.gpsimd.memset(padt[:, WC + 256 : WC + 260], PAD_EL)
            nc.sync.dma_start(out=g[NPOS : NPOS + 1, :], in_=padt[:])

            # ---- projection: g[q] = [ft|el|er](b0) | [ft|el|er](b1) ----
            hw = (nc.sync, nc.scalar)
            for b in range(B if stage in ('all', 'proj') else 0):
                for blk in range(NBLK):
                    j = b * NBLK + blk
                    ftb = pp.tile([DIN, QBLK], BF16, tag="ftb")
                    hw[j % 2].dma_start(
                        ftb[:],
                        featT[:, b * NPOS + blk * QBLK : b * NPOS + (blk + 1) * QBLK],
                    )
                    pw = pp.tile([P, 8, WC], BF16, tag="pw")
                    for i in range(8):
                        po = ppp.tile([P, WC], F32, tag="po")
                        nc.tensor.matmul(
                            po[:],
                            lhsT=ftb[:, i * P : (i + 1) * P],
                            rhs=w_sb[:],
                            start=True,
                            stop=True,
                        )
                        if i % 2 == 0:
                            nc.vector.tensor_copy(pw[:, i, :], po[:])
                        else:
                            nc.scalar.activation(pw[:, i, :], po[:], AF.Copy)
                    dst_ap = g[
                        blk * QBLK : (blk + 1) * QBLK, b * WC : (b + 1) * WC
                    ].rearrange("(i p) c -> p i c", p=P)
                    hw[(j + 1) % 2].dma_start(out=dst_ap, in_=pw[:])

            # ---- aggregation: per tile, slot-aligned segment softmax+sum ----
            off8 = 0
            ncall = 0
            for t in range(NT if stage in ('all', 'agg') else 0):
                Kt = K[t]
                KP = Kt + 1
                gt = ag.tile([P, KP1M, RW], BF16, tag="gt")
                # SWDGE descriptor ring holds ~1024 pairs: cap 8 chunks/call;
                # alternate the two SWDGE queues to overlap Q7 descriptor gen
                for g0 in range(0, KP, 8):
                    gn = min(8, KP - g0)
                    nc.gpsimd.dma_gather(
                        out_ap=gt[:, g0 : g0 + gn, :],
                        in_ap=g[:],
                        idxs_ap=idxs_res[:, off8 + g0 * 8 : off8 + (g0 + gn) * 8],
                        num_idxs=gn * P,
                        num_idxs_reg=gn * P,
                        elem_size=RW,
                        queue_num=ncall % NSWQ,
                    )
                    ncall += 1
                off8 += KP * 8

                # logits lg[p, k, (b,h)] = el_src + er_own (k-major: all the
                # downstream DVE/ACT ops stream contiguously)
                lg = sm.tile([P, KP1M, B * H], BF16, tag="lg")
                elv = gt[:, 0:Kt, 0 : 2 * WC].rearrange(
                    "p k (b r) -> p k b r", r=WC
                )[:, :, :, HD : HD + H]
                erv = gt[:, Kt, 0 : 2 * WC].rearrange("p (b r) -> p b r", r=WC)[
                    :, :, HD + H : HD + 2 * H
                ]
                lgv = lg[:, 0:Kt, :].rearrange("p k (b h) -> p k b h", h=H)
                nc.vector.tensor_tensor(
                    out=lgv,
                    in0=elv,
                    in1=erv[:, None, :, :].to_broadcast([P, Kt, B, H]),
                    op=OP.add,
                )
                l1 = sm.tile([P, KP1M, B * H], BF16, tag="l1")
                nc.vector.scalar_tensor_tensor(
                    out=l1[:, 0:Kt, :],
                    in0=lg[:, 0:Kt, :],
                    scalar=NEG,
                    in1=lg[:, 0:Kt, :],
                    op0=OP.mult,
                    op1=OP.max,
                )
                ex = sm.tile([P, KP1M, B * H], BF16, tag="ex")
                nc.scalar.activation(ex[:, 0:Kt, :], l1[:, 0:Kt, :], AF.Exp)

                # messages: ft *= ex (in-place, per batch)
                for b in range(B):
                    ftv = gt[:, 0:Kt, b * WC : b * WC + HD].rearrange(
                        "p k (h d) -> p k h d", d=D
                    )
                    exv = ex[:, 0:Kt, b * H : (b + 1) * H]
                    nc.vector.tensor_tensor(
                        out=ftv,
                        in0=ftv,
                        in1=exv[:, :, :, None].to_broadcast([P, Kt, H, D]),
                        op=OP.mult,
                    )

                # segment sum via identity matmuls accumulating in PSUM
                acc = psp.tile([P, FT], F32, tag="acc")
                for k in range(Kt):
                    rhs = gt[:, k, 0 : 2 * WC].rearrange("p (b r) -> p b r", r=WC)[
                        :, :, 0:HD
                    ]
                    nc.tensor.matmul(
                        acc[:],
                        lhsT=ident[:],
                        rhs=rhs,
                        start=(k == 0),
                        stop=(k == Kt - 1),
                    )

                dsum = sm.tile([P, B * H], F32, tag="dsum")
                nc.vector.tensor_reduce(
                    out=dsum[:],
                    in_=ex[:, 0:Kt, :].rearrange("p k q -> p q k"),
                    axis=mybir.AxisListType.X,
                    op=OP.add,
                )
                dse = sm.tile([P, B * H], F32, tag="dse")
                nc.vector.tensor_scalar_add(dse[:], dsum[:], 1e-20)
                rcp = sm.tile([P, B * H, 1], F32, tag="rcp")
                nc.vector.reciprocal(rcp[:, :, 0], dse[:])
                og = sm.tile([P, FT], F32, tag="og")
                if lrelu:
                    # fused finalize: og = Lrelu(acc * (1/den)) on ACT
                    for q in range(B * H):
                        nc.scalar.activation(
                            og[:, q * D : (q + 1) * D],
                            acc[:, q * D : (q + 1) * D],
                            AF.Lrelu,
                            scale=rcp[:, q, 0:1],
                            alpha=NEG,
                        )
                else:
                    o1 = sm.tile([P, B * H, D], F32, tag="o1")
                    nc.vector.tensor_tensor(
                        out=o1[:],
                        in0=acc[:].rearrange("p (q d) -> p q d", d=D),
                        in1=rcp[:].to_broadcast([P, B * H, D]),
                        op=OP.mult,
                    )
                    nc.vector.scalar_tensor_tensor(
                        out=og[:],
                        in0=o1[:].rearrange("p q d -> p (q d)"),
                        scalar=NEG,
                        in1=o1[:].rearrange("p q d -> p (q d)"),
                        op0=OP.mult,
                        op1=OP.max,
                    )
                nc.sync.dma_start(out=outd[t * P : (t + 1) * P, :], in_=og[:])

    nc.compile()
    return nc


def _make_inputs(feat, W, attn_l, attn_r, src, dst, n_nodes=N, n_cores=NCORES):
    feat = np.asarray(feat, dtype=np.float32)
    W = np.asarray(W, dtype=np.float32)
    attn_l = np.asarray(attn_l, dtype=np.float32)
    attn_r = np.asarray(attn_r, dtype=np.float32)
    src = np.asarray(src)
    dst = np.asarray(dst)

    K, idxs_all, pos2node = _host_prep(src, dst)

    real = pos2node >= 0
    ftp = np.zeros((B, NPOS, DIN), np.float32)
    ftp[:, real, :] = feat[pos2node[real]].transpose(1, 0, 2)
    featT = np.ascontiguousarray(ftp.reshape(B * NPOS, DIN).T).astype(
        ml_dtypes.bfloat16
    )
    Wl = (W.reshape(DIN, H, D) * attn_l[None]).sum(-1)
    Wr = (W.reshape(DIN, H, D) * attn_r[None]).sum(-1)
    wmat = np.concatenate([W, Wl, Wr], axis=1).astype(ml_dtypes.bfloat16)

    in_maps = [
        {"featT": featT, "wmat": wmat, "idxs": idxs_all[c]} for c in range(n_cores)
    ]
    return K, in_maps, pos2node


_CACHE = {}


def kernel(feat, W, attn_l, attn_r, src, dst):
    K, in_maps, pos2node = _make_inputs(feat, W, attn_l, attn_r, src, dst)
    key = tuple(K)
    if key not in _CACHE:
        _CACHE[key] = _build(K)
    nc = _CACHE[key]
    res = run_bass_kernel_spmd(nc, in_maps, list(range(NCORES))).results
    out = np.empty((N, B, H, D), np.float32)
    for c in range(NCORES):
        nodes = pos2node[c * NPOSC : c * NPOSC + NPC]
        out[nodes] = res[c]["out"][:NPC].reshape(NPC, B, H, D)
    return out


if __name__ == "__main__":
    rng = np.random.default_rng(0)
    feat = rng.standard_normal((N, B, DIN), dtype=np.float32)
    W = rng.standard_normal((DIN, H * D), dtype=np.float32) / np.sqrt(DIN)
    al = rng.standard_normal((H, D), dtype=np.float32) * 0.1
    ar = rng.standard_normal((H, D), dtype=np.float32) * 0.1
    src = rng.integers(0, N, E).astype(np.int32)
    dst = rng.integers(0, N, E).astype(np.int32)
    out = kernel(feat=feat, W=W, attn_l=al, attn_r=ar, src=src, dst=dst)
    print(out.shape, out.dtype, np.abs(out).mean())


# revision 14
# speedup vs baseline: 1.9537x; 1.0281x over previous
"""Trainium2 Bass kernel for BatchGATConv (GAT message passing).

Strategy (8 NeuronCores, SPMD, dst-partitioned):
  - Host: in-degree-sort each core's 2500 dst nodes into 20 tiles of 128 so
    every tile has near-uniform degree; chunk k of a tile holds the k-th
    in-edge of each of the tile's 128 nodes (slot-aligned, so dst-local ==
    partition and no one-hot matmuls are needed). Padded slots point at a
    dedicated pad row whose attention logit is -150 (=> exp == 0).
  - Projection (replicated on all cores, bf16): g[pos] row pair
    [ft(b0)|el|er | ft(b1)|el|er | pad] with attn_l/attn_r folded into the
    weight matrix on the host; 1024-row blocks, batched DMAs.
  - Aggregation per tile: ONE dma_gather fetches all (K+1)*128 row pairs
    (the +1 chunk gathers the tile's own rows for er_dst); logits
    lg = el_src + er_dst, leaky, exp on ACT; messages ft*ex scaled in-place
    on DVE; PSUM-accumulated via identity matmuls (per-partition segment sum);
    denominator via DVE reduce; out = leaky(num/den).
"""

import numpy as np

try:
    import concourse.bass as bass
except ImportError:  # pragma: no cover
    import sys

    sys.path.insert(0, "/opt/trn_rl_repo")
    import concourse.bass as bass

import ml_dtypes
import concourse.bacc as bacc
import concourse.mybir as mybir
import concourse.tile as tile
from concourse.bass_utils import run_bass_kernel_spmd

P = 128
F32 = mybir.dt.float32
BF16 = mybir.dt.bfloat16
I32 = mybir.dt.int32
I16 = mybir.dt.int16

# problem constants
N, B, DIN, H, D, E = 20000, 2, 128, 4, 64, 320000
NEG = 0.2
NCORES = 8
NPC = N // NCORES  # 2500 dst nodes per core
NT = 20  # node tiles per core
NPOSC = NT * P  # 2560 positions per core (60 pad positions)
NPOS = NCORES * NPOSC  # 20480 global positions
PADROW = NPOS  # pad pair row index
GROWS = NPOS + 2
WC = 264  # per-(n,b) block: [ft 256 | el 4 | er 4]
HD = H * D  # 256
FT = B * HD  # 512
RW = 640  # g row width (2*WC + 112 pad; 1280B, %256 for dma_gather)
PAD_EL = -150.0
QBLK = 2048  # projection rows per block
NBLK = NPOS // QBLK  # 10 per batch
NSWQ = 4  # SWDGE queues (Q7 descriptor-gen parallelism)


def _host_prep(src, dst):
    """Degree-sort nodes per core; build per-core wrapped int16 gather tables.

    Returns (K, idxs_all, pos2node):
      K: per-tile chunk count, shared across cores (max in-degree in tile)
      idxs_all[c]: [16, sum((K+1)*8)] int16 wrapped gather indices
      pos2node: [NPOS] original node id per position (-1 = pad)
    """
    deg = np.bincount(dst, minlength=N).astype(np.int64)
    pos2node = np.full(NPOS, -1, np.int64)
    for c in range(NCORES):
        ids = np.arange(c * NPC, (c + 1) * NPC)
        order = np.argsort(-deg[ids], kind="stable")
        pos2node[c * NPOSC : c * NPOSC + NPC] = ids[order]
    node2pos = np.empty(N, np.int64)
    real = pos2node >= 0
    node2pos[pos2node[real]] = np.nonzero(real)[0]

    first = np.arange(NCORES)[:, None] * NPOSC + np.arange(NT)[None, :] * P
    K = np.maximum(deg[pos2node[first]].max(axis=0), 1)  # [NT]

    dpos = node2pos[dst]
    spos = node2pos[src]
    order_e = np.argsort(dpos, kind="stable")
    ds = dpos[order_e]
    ss = spos[order_e]
    starts = np.searchsorted(ds, np.arange(NPOS + 1))
    k_e = np.arange(E) - starts[ds]

    idxs_all = []
    for c in range(NCORES):
        cols = []
        for t in range(NT):
            Kt = int(K[t])
            base = c * NPOSC + t * P
            sl = np.full((Kt + 1) * P, PADROW, np.int64)
            e0, e1 = starts[base], starts[base + P]
            pp_ = ds[e0:e1] - base
            sl[k_e[e0:e1] * P + pp_] = ss[e0:e1]
            sl[Kt * P : (Kt + 1) * P] = base + np.arange(P)
            cols.append(sl.reshape(-1, 16).T)
        idxs_all.append(
            np.ascontiguousarray(
                np.tile(np.concatenate(cols, axis=1), (8, 1))
            ).astype(np.int16)
        )
    return list(map(int, K)), idxs_all, pos2node


def _build(K, stage='all', lrelu=False):
    S8 = sum((k + 1) * 8 for k in K)
    KP1M = max(K) + 1
    AF = mybir.ActivationFunctionType
    OP = mybir.AluOpType

    nc = bacc.Bacc(trn_type="TRN2", num_swdge_queues=NSWQ)
    featT = nc.dram_tensor("featT", [DIN, B * NPOS], BF16, kind="ExternalInput")
    wmat = nc.dram_tensor("wmat", [DIN, WC], BF16, kind="ExternalInput")
    idxsd = nc.dram_tensor("idxs", [128, S8], I16, kind="ExternalInput")
    outd = nc.dram_tensor("out", [NPOSC, FT], F32, kind="ExternalOutput")
    g = nc.dram_tensor("gtab", [GROWS, RW], BF16)

    with tile.TileContext(nc) as tc:
        with (
            tc.tile_pool(name="const", bufs=1) as cp,
            tc.tile_pool(name="proj", bufs=3) as pp,
            tc.tile_pool(name="projps", bufs=6, space="PSUM") as ppp,
            tc.tile_pool(name="agg", bufs=3) as ag,
            tc.tile_pool(name="small", bufs=4) as sm,
            tc.tile_pool(name="accps", bufs=2, space="PSUM") as psp,
        ):
            # resident constants
            w_sb = cp.tile([DIN, WC], BF16)
            nc.sync.dma_start(w_sb[:], wmat[:])
            idxs_res = cp.tile([128, S8], I16)
            nc.sync.dma_start(idxs_res[:], idxsd[:])
            iota_r = cp.tile([P, P], I32)
            nc.gpsimd.iota(iota_r[:], pattern=[[1, P]], base=0, channel_multiplier=0)
            iota_rf = cp.tile([P, P], F32)
            nc.vector.tensor_copy(iota_rf[:], iota_r[:])
            iota_p = cp.tile([P, 1], I32)
            nc.gpsimd.iota(iota_p[:], pattern=[[1, 1]], base=0, channel_multiplier=1)
            iota_pf = cp.tile([P, 1], F32)
            nc.vector.tensor_copy(iota_pf[:], iota_p[:])
            ident = cp.tile([P, P], BF16)
            nc.vector.tensor_scalar(
                out=ident[:],
                in0=iota_rf[:],
                scalar1=iota_pf[:, 0:1],
                scalar2=None,
                op0=OP.is_equal,
            )
            # pad row: ft/er = 0, el = PAD_EL -> exp(leaky(lg)) == 0
            padt = cp.tile([1, RW], BF16)
            nc.gpsimd.memset(padt[:], 0.0)
            nc.gpsimd.memset(padt[:, 256:260], PAD_EL)
            nc.gpsimd.memset(padt[:, WC + 256 : WC + 260], PAD_EL)
            nc.sync.dma_start(out=g[NPOS : NPOS + 1, :], in_=padt[:])

            # ---- projection: g[q] = [ft|el|er](b0) | [ft|el|er](b1) ----
            hw = (nc.sync, nc.scalar)
            for b in range(B if stage in ('all', 'proj') else 0):
                for blk in range(NBLK):
                    j = b * NBLK + blk
                    ftb = pp.tile([DIN, QBLK], BF16, tag="ftb")
                    hw[j % 2].dma_start(
                        ftb[:],
                        featT[:, b * NPOS + blk * QBLK : b * NPOS + (blk + 1) * QBLK],
                    )
                    pw = pp.tile([P, QBLK // P, WC], BF16, tag="pw")
                    for i in range(QBLK // P):
                        po = ppp.tile([P, WC], F32, tag="po")
                        nc.tensor.matmul(
                            po[:],
                            lhsT=ftb[:, i * P : (i + 1) * P],
                            rhs=w_sb[:],
                            start=True,
                            stop=True,
                        )
                        if i % 2 == 0:
                            nc.vector.tensor_copy(pw[:, i, :], po[:])
                        else:
                            nc.scalar.activation(pw[:, i, :], po[:], AF.Copy)
                    dst_ap = g[
                        blk * QBLK : (blk + 1) * QBLK, b * WC : (b + 1) * WC
                    ].rearrange("(i p) c -> p i c", p=P)
                    hw[(j + 1) % 2].dma_start(out=dst_ap, in_=pw[:])

            # ---- aggregation: per tile, slot-aligned segment softmax+sum ----
            off8 = 0
            ncall = 0
            for t in range(NT if stage in ('all', 'agg') else 0):
                Kt = K[t]
                KP = Kt + 1
                gt = ag.tile([P, KP1M, RW], BF16, tag="gt")
                # SWDGE descriptor ring holds ~1024 pairs: cap 8 chunks/call;
                # alternate the two SWDGE queues to overlap Q7 descriptor gen
                for g0 in range(0, KP, 8):
                    gn = min(8, KP - g0)
                    nc.gpsimd.dma_gather(
                        out_ap=gt[:, g0 : g0 + gn, :],
                        in_ap=g[:],
                        idxs_ap=idxs_res[:, off8 + g0 * 8 : off8 + (g0 + gn) * 8],
                        num_idxs=gn * P,
                        num_idxs_reg=gn * P,
                        elem_size=RW,
                        queue_num=ncall % NSWQ,
                    )
                    ncall += 1
                off8 += KP * 8

                # logits lg[p, k, (b,h)] = el_src + er_own (k-major: all the
                # downstream DVE/ACT ops stream contiguously)
                lg = sm.tile([P, KP1M, B * H], BF16, tag="lg")
                elv = gt[:, 0:Kt, 0 : 2 * WC].rearrange(
                    "p k (b r) -> p k b r", r=WC
                )[:, :, :, HD : HD + H]
                erv = gt[:, Kt, 0 : 2 * WC].rearrange("p (b r) -> p b r", r=WC)[
                    :, :, HD + H : HD + 2 * H
                ]
                lgv = lg[:, 0:Kt, :].rearrange("p k (b h) -> p k b h", h=H)
                nc.vector.tensor_tensor(
                    out=lgv,
                    in0=elv,
                    in1=erv[:, None, :, :].to_broadcast([P, Kt, B, H]),
                    op=OP.add,
                )
                l1 = sm.tile([P, KP1M, B * H], BF16, tag="l1")
                nc.vector.scalar_tensor_tensor(
                    out=l1[:, 0:Kt, :],
                    in0=lg[:, 0:Kt, :],
                    scalar=NEG,
                    in1=lg[:, 0:Kt, :],
                    op0=OP.mult,
                    op1=OP.max,
                )
                ex = sm.tile([P, KP1M, B * H], BF16, tag="ex")
                nc.scalar.activation(ex[:, 0:Kt, :], l1[:, 0:Kt, :], AF.Exp)

                # messages: ft *= ex (in-place, per batch)
                for b in range(B):
                    ftv = gt[:, 0:Kt, b * WC : b * WC + HD].rearrange(
                        "p k (h d) -> p k h d", d=D
                    )
                    exv = ex[:, 0:Kt, b * H : (b + 1) * H]
                    nc.vector.tensor_tensor(
                        out=ftv,
                        in0=ftv,
                        in1=exv[:, :, :, None].to_broadcast([P, Kt, H, D]),
                        op=OP.mult,
                    )

                # segment sum via identity matmuls accumulating in PSUM
                acc = psp.tile([P, FT], F32, tag="acc")
                for k in range(Kt):
                    rhs = gt[:, k, 0 : 2 * WC].rearrange("p (b r) -> p b r", r=WC)[
                        :, :, 0:HD
                    ]
                    nc.tensor.matmul(
                        acc[:],
                        lhsT=ident[:],
                        rhs=rhs,
                        start=(k == 0),
                        stop=(k == Kt - 1),
                    )

                dsum = sm.tile([P, B * H], F32, tag="dsum")
                nc.vector.tensor_reduce(
                    out=dsum[:],
                    in_=ex[:, 0:Kt, :].rearrange("p k q -> p q k"),
                    axis=mybir.AxisListType.X,
                    op=OP.add,
                )
                dse = sm.tile([P, B * H], F32, tag="dse")
                nc.vector.tensor_scalar_add(dse[:], dsum[:], 1e-20)
                rcp = sm.tile([P, B * H, 1], F32, tag="rcp")
                nc.vector.reciprocal(rcp[:, :, 0], dse[:])
                og = sm.tile([P, FT], F32, tag="og")
                if lrelu:
                    # fused finalize: og = Lrelu(acc * (1/den)) on ACT
                    for q in range(B * H):
                        nc.scalar.activation(
                            og[:, q * D : (q + 1) * D],
                            acc[:, q * D : (q + 1) * D],
                            AF.Lrelu,
                            scale=rcp[:, q, 0:1],
                            alpha=NEG,
                        )
                else:
                    o1 = sm.tile([P, B * H, D], F32, tag="o1")
                    nc.vector.tensor_tensor(
                        out=o1[:],
                        in0=acc[:].rearrange("p (q d) -> p q d", d=D),
                        in1=rcp[:].to_broadcast([P, B * H, D]),
                        op=OP.mult,
                    )
                    nc.vector.scalar_tensor_tensor(
                        out=og[:],
                        in0=o1[:].rearrange("p q d -> p (q d)"),
                        scalar=NEG,
                        in1=o1[:].rearrange("p q d -> p (q d)"),
                        op0=OP.mult,
                        op1=OP.max,
                    )
                nc.sync.dma_start(out=outd[t * P : (t + 1) * P, :], in_=og[:])

    nc.compile()
    return nc


def _make_inputs(feat, W, attn_l, attn_r, src, dst, n_nodes=N, n_cores=NCORES):
    feat = np.asarray(feat, dtype=np.float32)
    W = np.asarray(W, dtype=np.float32)
    attn_l = np.asarray(attn_l, dtype=np.float32)
    attn_r = np.asarray(attn_r, dtype=np.float32)
    src = np.asarray(src)
    dst = np.asarray(dst)

    K, idxs_all, pos2node = _host_prep(src, dst)

    real = pos2node >= 0
    ftp = np.zeros((B, NPOS, DIN), np.float32)
    ftp[:, real, :] = feat[pos2node[real]].transpose(1, 0, 2)
    featT = np.ascontiguousarray(ftp.reshape(B * NPOS, DIN).T).astype(
        ml_dtypes.bfloat16
    )
    Wl = (W.reshape(DIN, H, D) * attn_l[None]).sum(-1)
    Wr = (W.reshape(DIN, H, D) * attn_r[None]).sum(-1)
    wmat = np.concatenate([W, Wl, Wr], axis=1).astype(ml_dtypes.bfloat16)

    in_maps = [
        {"featT": featT, "wmat": wmat, "idxs": idxs_all[c]} for c in range(n_cores)
    ]
    return K, in_maps, pos2node


_CACHE = {}


def kernel(feat, W, attn_l, attn_r, src, dst):
    K, in_maps, pos2node = _make_inputs(feat, W, attn_l, attn_r, src, dst)
    key = tuple(K)
    if key not in _CACHE:
        _CACHE[key] = _build(K)
    nc = _CACHE[key]
    res = run_bass_kernel_spmd(nc, in_maps, list(range(NCORES))).results
    out = np.empty((N, B, H, D), np.float32)
    for c in range(NCORES):
        nodes = pos2node[c * NPOSC : c * NPOSC + NPC]
        out[nodes] = res[c]["out"][:NPC].reshape(NPC, B, H, D)
    return out


if __name__ == "__main__":
    rng = np.random.default_rng(0)
    feat = rng.standard_normal((N, B, DIN), dtype=np.float32)
    W = rng.standard_normal((DIN, H * D), dtype=np.float32) / np.sqrt(DIN)
    al = rng.standard_normal((H, D), dtype=np.float32) * 0.1
    ar = rng.standard_normal((H, D), dtype=np.float32) * 0.1
    src = rng.integers(0, N, E).astype(np.int32)
    dst = rng.integers(0, N, E).astype(np.int32)
    out = kernel(feat=feat, W=W, attn_l=al, attn_r=ar, src=src, dst=dst)
    print(out.shape, out.dtype, np.abs(out).mean())
